# revision 14
# baseline (speedup 1.0000x reference)
"""LSNN layer forward on 8 Trainium2 NeuronCores (data-parallel over batch).

Reference math (per batch row):
    L1    = x_t @ W_syn.T + b_syn
    alpha = sigmoid((L1 + u_t) @ W_Tm.T + b_Tm)
    rho   = sigmoid((L1 + b_t) @ W_Tadp.T + b_Tadp)
    b_new = rho * b_t + (1 - rho) * spk
    thr   = 0.01 + 1.8 * b_new
    u_new = u_t + (L1 - u_t) / alpha
    o_spk = (u_new - thr > 0) as f32

Device formulation (activations transposed [neuron, batch], all values
pre-scaled by SA=32 so the fp8 moving operand uses the e4m3 range):
    u_new - thr > 0  <=>  alpha*(u_new - thr) > 0   (alpha > 0)
    d = SA*(L1-u) + (csd + rho*t2d18) * alpha  > 0
        csd   = SA*(u - 1.8*spk - 0.01)
        t2d18 = -1.8*SA*(b - spk)
Both branches use the Sigmoid activation (no Exp) so the scalar engine
loads its activation table once.  The kernel ships the raw d scores as
bf16; the host applies (d > 0) when unpacking (sign is preserved
exactly by the bf16 downcast).

The sigmoid matmuls run in fp8e4m3 DoubleRow mode (2 contraction rows
per cycle, 2x f32r throughput): weights scaled by SW=2048, moving
operand z = SA*(L1+state) quantized on the fly by the DVE; psum is
descaled by 1/(SA*SW) = 2^-16 inside the sigmoid's activation scale.
Near the spike threshold the sigmoid-branch error is damped by
|d(u_new)/dz| <= 0.01, so fp8 there costs only ~50 extra spike flips
(budget ~1670 at rel-err 2e-2; measured v2: 368 flips total).

mm1 (L1 = W_syn @ x) precision modes (MM1_MODE env):
    f32r - TF32-like (1 cyc/row), ~1.5e-4 rel err on L1, 16MB weights
    fp16 - fp16 (1 cyc/row), ~4e-4 rel err on L1, 8MB weights

Sharding: batch 4096 -> 8 shards of 512; weights replicated; no
cross-core communication.  State tensors ship as bf16 combinations,
exact for this problem's zero-filled states.
"""

import os

import numpy as np
import ml_dtypes

import concourse.bacc as bacc
import concourse.tile as tile
import concourse.mybir as mybir
from concourse.bass_utils import run_bass_kernel_spmd

AF = mybir.ActivationFunctionType
ALU = mybir.AluOpType
PM = mybir.MatmulPerfMode

B, I, O = 4096, 2048, 2048
NCORES = 8
BC = B // NCORES          # 512 batch rows per core
P = 128                   # partitions
KT = I // P               # 16 k-tiles (mm1)
OT = O // P               # 16 output neuron tiles
KK = O // 256             # 8 double-row k-tiles (mm2/mm3)

SA = 32.0                 # global activation scale
SW = 2048.0               # fp8 weight scale
PS_INV = 1.0 / (SA * SW)  # 2^-16, exact
FP8_MAX = 240.0           # ml_dtypes.float8_e4m3 max finite

F32 = mybir.dt.float32
F32R = mybir.dt.float32r
F16 = mybir.dt.float16
BF16 = mybir.dt.bfloat16
FP8 = mybir.dt.float8e4
U8 = mybir.dt.uint8

MM1_MODE = os.environ.get("MM1_MODE", "fp16")

# phase-2 o-tile groups: pairs for wide (1024-col) DVE ops, singles at
# the end to keep the exposed pointwise tail after the last matmul short
GROUPS = [(0, 1), (2, 3), (4, 5), (6, 7), (8, 9), (10, 11), (12, 13), (14,), (15,)]


def build_nc():
    mm1_dt = {"f32r": F32R, "fp16": F16}[MM1_MODE]

    nc = bacc.Bacc("TRN2", target_bir_lowering=False, debug=False)

    x_d = nc.dram_tensor("xh", (P, KT, BC), mm1_dt, kind="ExternalInput").ap()
    u32_d = nc.dram_tensor("u32", (P, OT, BC), BF16, kind="ExternalInput").ap()
    b32_d = nc.dram_tensor("b32", (P, OT, BC), BF16, kind="ExternalInput").ap()
    t2d_d = nc.dram_tensor("t2d", (P, OT, BC), BF16, kind="ExternalInput").ap()
    csd_d = nc.dram_tensor("csd", (P, OT, BC), BF16, kind="ExternalInput").ap()
    wsyn_d = nc.dram_tensor("wsyn", (P, OT, KT, P), mm1_dt, kind="ExternalInput").ap()
    wtm_d = nc.dram_tensor("wtm", (P, OT, KK, 2, P), FP8, kind="ExternalInput").ap()
    wtadp_d = nc.dram_tensor("wtadp", (P, OT, KK, 2, P), FP8, kind="ExternalInput").ap()
    bsyn_d = nc.dram_tensor("bsyn", (P, OT), F32, kind="ExternalInput").ap()
    btm_d = nc.dram_tensor("btm", (P, OT), F32, kind="ExternalInput").ap()
    btadp_d = nc.dram_tensor("btadp", (P, OT), F32, kind="ExternalInput").ap()
    out_d = nc.dram_tensor("out", (P, OT, BC), BF16, kind="ExternalOutput").ap()

    with tile.TileContext(nc) as tc:
        with (
            tc.tile_pool(name="persist", bufs=1) as persist,
            tc.tile_pool(name="wpool", bufs=5) as wpool,
            tc.tile_pool(name="wqpool", bufs=8) as wqpool,
            tc.tile_pool(name="iopool", bufs=4) as iopool,
            tc.tile_pool(name="tmpf", bufs=4) as tmpf,
            tc.tile_pool(name="tmpb", bufs=6) as tmpb,
            tc.tile_pool(name="outp", bufs=3) as outp,
            tc.tile_pool(name="psum", bufs=4, space="PSUM") as psum,
        ):
            xsb = persist.tile([P, KT, BC], mm1_dt, tag="xsb")
            l1s = persist.tile([P, OT, BC], F32, tag="l1s")
            z1q = persist.tile([P, OT, BC], FP8, tag="z1q")
            z2q = persist.tile([P, OT, BC], FP8, tag="z2q")
            u32 = persist.tile([P, OT, BC], BF16, tag="u32")
            t1s = persist.tile([P, OT, BC], F32, tag="t1s")
            bsyn = persist.tile([P, OT], F32, tag="bsyn")
            btm = persist.tile([P, OT], F32, tag="btm")
            btadp = persist.tile([P, OT], F32, tag="btadp")

            # startup: interleave weight-tile-0 k-slices with x k-slices so
            # the first matmul chain starts after ~128KB has landed
            w0 = wpool.tile([P, KT, P], mm1_dt, tag="w")
            qh = KT // 8
            nc.sync.dma_start(w0[:, :qh, :], wsyn_d[:, 0, :qh, :])
            nc.scalar.dma_start(xsb[:, :qh, :], x_d[:, :qh, :])
            nc.sync.dma_start(w0[:, qh:, :], wsyn_d[:, 0, qh:, :])
            for c in range(1, 8):
                nc.scalar.dma_start(xsb[:, c * qh:(c + 1) * qh, :],
                                    x_d[:, c * qh:(c + 1) * qh, :])
            nc.gpsimd.dma_start(bsyn[:], bsyn_d[:])
            nc.gpsimd.dma_start(btm[:], btm_d[:])
            nc.gpsimd.dma_start(btadp[:], btadp_d[:])

            # ---- phase 1: L1 = W_syn @ x; z1 = SA*(L1+u), z2 = SA*(L1+b) fp8
            # State DMAs (u32/b32) are deferred past t=4 so the DMA-engine
            # ramp-up window feeds only weights+x (early chains were
            # DMA-starved).  z-adds for pairs 0-2 run at t=5; pair p>=3 at
            # t=2p+1.  Each pair computes z2q first so phase 2 (whose first
            # chains read z2q) is gated as little as possible by pair 7.
            h = KT // 2
            bquads = {}

            def issue_states(q):
                lo = 4 * q
                nc.gpsimd.dma_start(u32[:, lo:lo + 4, :], u32_d[:, lo:lo + 4, :])
                bquads[q] = iopool.tile([P, 4, BC], BF16, tag="io", name="bq")
                nc.gpsimd.dma_start(bquads[q][:], b32_d[:, lo:lo + 4, :])

            def zadds(p):
                lo, hi = 2 * p, 2 * p + 2
                bq = bquads[p // 2][:, (p % 2) * 2:(p % 2) * 2 + 2, :]
                nc.vector.tensor_add(z2q[:, lo:hi, :], l1s[:, lo:hi, :], bq)
                nc.vector.tensor_add(z1q[:, lo:hi, :], l1s[:, lo:hi, :],
                                     u32[:, lo:hi, :])
                nc.vector.tensor_sub(t1s[:, lo:hi, :], l1s[:, lo:hi, :],
                                     u32[:, lo:hi, :])

            for t in range(OT):
                if t == 0:
                    w = w0
                else:
                    w = wpool.tile([P, KT, P], mm1_dt, tag="w")
                    nc.sync.dma_start(w[:], wsyn_d[:, t])
                if t == 4:
                    issue_states(0)
                elif t == 6:
                    issue_states(1)
                elif t == 9:
                    issue_states(2)
                elif t == 12:
                    issue_states(3)
                if t == 10:
                    # prefetch the first two phase-2 fp8 weight groups
                    wq_pre = {}
                    for tt in (0, 1, 2, 3):
                        wr_p = wqpool.tile([P, KK, 2, P], FP8, tag="wq")
                        nc.gpsimd.dma_start(wr_p[:], wtadp_d[:, tt])
                        wa_p = wqpool.tile([P, KK, 2, P], FP8, tag="wq")
                        nc.gpsimd.dma_start(wa_p[:], wtm_d[:, tt])
                        wq_pre[tt] = (wr_p, wa_p)
                ps = psum.tile([P, 2, BC], F32, tag="ps")
                for k in range(KT):
                    nc.tensor.matmul(ps[:, 0, :], w[:, k, :], xsb[:, k, :],
                                     start=(k == 0), stop=(k == KT - 1))
                nc.scalar.activation(l1s[:, t, :], ps[:, 0, :], AF.Identity,
                                     bias=bsyn[:, t:t + 1], scale=SA)
                if t == 5:
                    zadds(0), zadds(1)
                elif t == 7:
                    zadds(2), zadds(3)
                elif t % 2 == 1 and t >= 9:
                    zadds((t - 1) // 2)

            # ---- phase 2: sigmoid branches (fp8 DoubleRow) + pointwise tail
            for g in GROUPS:
                gw = len(g)
                t0 = g[0]
                ws = {}
                for t in g:
                    if t in wq_pre:
                        ws[t] = wq_pre[t]
                    else:
                        wr = wqpool.tile([P, KK, 2, P], FP8, tag="wq")
                        nc.gpsimd.dma_start(wr[:], wtadp_d[:, t])
                        wa = wqpool.tile([P, KK, 2, P], FP8, tag="wq")
                        nc.gpsimd.dma_start(wa[:], wtm_d[:, t])
                        ws[t] = (wr, wa)
                t2t = iopool.tile([P, 2, BC], BF16, tag="io", name="t2t")[:, :gw, :]
                cst = iopool.tile([P, 2, BC], BF16, tag="io", name="cst")[:, :gw, :]
                nc.gpsimd.dma_start(t2t[:], t2d_d[:, t0:t0 + gw, :])
                nc.gpsimd.dma_start(cst[:], csd_d[:, t0:t0 + gw, :])

                t1 = t1s[:, t0:t0 + gw, :]

                psr = psum.tile([P, 2, BC], F32, tag="ps")
                for j, t in enumerate(g):
                    for k in range(KK):
                        nc.tensor.matmul(psr[:, j, :], ws[t][0][:, k, :, :],
                                         z2q[:, 2 * k:2 * k + 2, :],
                                         start=(k == 0), stop=(k == KK - 1),
                                         perf_mode=PM.DoubleRow)
                rho = tmpb.tile([P, 2, BC], BF16, tag="t", name="rho")[:, :gw, :]
                for j, t in enumerate(g):
                    nc.scalar.activation(rho[:, j, :], psr[:, j, :], AF.Sigmoid,
                                         bias=btadp[:, t:t + 1], scale=PS_INV)
                m2 = tmpb.tile([P, 2, BC], BF16, tag="t", name="m2")[:, :gw, :]
                nc.vector.tensor_mul(m2[:], rho[:], t2t[:])
                q = tmpb.tile([P, 2, BC], BF16, tag="t", name="q")[:, :gw, :]
                nc.vector.tensor_add(q[:], m2[:], cst[:])

                psa = psum.tile([P, 2, BC], F32, tag="ps")
                for j, t in enumerate(g):
                    for k in range(KK):
                        nc.tensor.matmul(psa[:, j, :], ws[t][1][:, k, :, :],
                                         z1q[:, 2 * k:2 * k + 2, :],
                                         start=(k == 0), stop=(k == KK - 1),
                                         perf_mode=PM.DoubleRow)
                alpha = tmpf.tile([P, 2, BC], F32, tag="t", name="alpha")[:, :gw, :]
                r = tmpf.tile([P, 2, BC], F32, tag="t", name="r")[:, :gw, :]
                dd = outp.tile([P, 2, BC], BF16, tag="o", name="dd")[:, :gw, :]
                if g is GROUPS[-1]:
                    # last tile: pipeline the exposed tail in half-columns
                    hb = BC // 2
                    for s in (slice(0, hb), slice(hb, BC)):
                        nc.scalar.activation(alpha[:, 0, s], psa[:, 0, s],
                                             AF.Sigmoid, bias=btm[:, t0:t0 + 1],
                                             scale=PS_INV)
                        nc.vector.tensor_mul(r[:, 0, s], q[:, 0, s], alpha[:, 0, s])
                        nc.vector.tensor_add(dd[:, 0, s], t1[:, 0, s], r[:, 0, s])
                        nc.sync.dma_start(out_d[:, t0, s], dd[:, 0, s])
                else:
                    for j, t in enumerate(g):
                        nc.scalar.activation(alpha[:, j, :], psa[:, j, :], AF.Sigmoid,
                                             bias=btm[:, t:t + 1], scale=PS_INV)
                    nc.vector.tensor_mul(r[:], q[:], alpha[:])
                    nc.vector.tensor_add(dd[:], t1[:], r[:])
                    nc.sync.dma_start(out_d[:, t0:t0 + gw, :], dd[:])

    nc.compile()
    return nc


def _pack_w1(w: np.ndarray) -> np.ndarray:
    # [O, I] -> [p, t, k, m] with w[t*128+m, k*128+p] at [p, t, k, m]
    return np.ascontiguousarray(w.reshape(OT, P, KT, P).transpose(3, 0, 2, 1))


def _pack_wq(w: np.ndarray) -> np.ndarray:
    # [O, O] -> [p, t, kk, i, m] fp8 with w[t*128+m, kk*256+i*128+p]*SW
    ws = np.clip(np.asarray(w, np.float64) * SW, -FP8_MAX, FP8_MAX)
    a = ws.reshape(OT, P, KK, 2, P).transpose(4, 0, 2, 3, 1)
    return np.ascontiguousarray(a.astype(ml_dtypes.float8_e4m3))


def _pack_bias(v: np.ndarray) -> np.ndarray:
    return np.ascontiguousarray(np.asarray(v, np.float32).reshape(OT, P).T)


def _pack_state(a: np.ndarray) -> np.ndarray:
    # [BC, O] -> [P, OT, BC] bf16
    return np.ascontiguousarray(
        a.reshape(BC, OT, P).transpose(2, 1, 0).astype(ml_dtypes.bfloat16))


def prepare_in_maps(x_t, u_t, b_t, spk, W_syn, b_syn, W_Tm, b_Tm, W_Tadp, b_Tadp):
    np1 = {"f32r": np.float32, "fp16": np.float16}[MM1_MODE]
    wsyn = _pack_w1(np.asarray(W_syn, np.float32)).astype(np1)
    wtm = _pack_wq(W_Tm)
    wtadp = _pack_wq(W_Tadp)
    bsyn = _pack_bias(SA * np.asarray(b_syn, np.float32))
    btm = _pack_bias(b_Tm)
    btadp = _pack_bias(b_Tadp)

    in_maps = []
    for c in range(NCORES):
        sl = slice(c * BC, (c + 1) * BC)
        xc = np.asarray(x_t[sl], np.float32)
        xp = np.ascontiguousarray(
            xc.reshape(BC, KT, P).transpose(2, 1, 0)).astype(np1)
        uc = np.asarray(u_t[sl], np.float64)
        bc = np.asarray(b_t[sl], np.float64)
        sc = np.asarray(spk[sl], np.float64)
        m = {
            "xh": xp,
            "u32": _pack_state(SA * uc),
            "b32": _pack_state(SA * bc),
            "t2d": _pack_state(-1.8 * SA * (bc - sc)),
            "csd": _pack_state(SA * (uc - 1.8 * sc - 0.01)),
            "wsyn": wsyn, "wtm": wtm, "wtadp": wtadp,
            "bsyn": bsyn, "btm": btm, "btadp": btadp,
        }
        in_maps.append(m)
    return in_maps


def unpack_output(results) -> np.ndarray:
    # per-core out: [P, OT, BC] bf16 scores -> spikes [BC, O] f32
    parts = []
    for r in results:
        d = r["out"].astype(np.float32).transpose(2, 1, 0).reshape(BC, O)
        parts.append((d > 0).astype(np.float32))
    return np.ascontiguousarray(np.concatenate(parts, axis=0))


_NC = None


def get_nc():
    global _NC
    if _NC is None:
        _NC = build_nc()
    return _NC


def run_sharded(in_maps, trace=False, **kw):
    nc = get_nc()
    return run_bass_kernel_spmd(nc, in_maps, list(range(NCORES)), trace=trace, **kw)


def kernel(**inputs) -> np.ndarray:
    in_maps = prepare_in_maps(**inputs)
    res = run_sharded(in_maps)
    return unpack_output(res.results)


# revision 15
# speedup vs baseline: 1.0732x; 1.0732x over previous
"""LSNN layer forward on 8 Trainium2 NeuronCores (data-parallel over batch).

Reference math (per batch row):
    L1    = x_t @ W_syn.T + b_syn
    alpha = sigmoid((L1 + u_t) @ W_Tm.T + b_Tm)
    rho   = sigmoid((L1 + b_t) @ W_Tadp.T + b_Tadp)
    b_new = rho * b_t + (1 - rho) * spk
    thr   = 0.01 + 1.8 * b_new
    u_new = u_t + (L1 - u_t) / alpha
    o_spk = (u_new - thr > 0) as f32

Device formulation (activations transposed [neuron, batch], all values
pre-scaled by SA=32 so the fp8 moving operand uses the e4m3 range):
    u_new - thr > 0  <=>  alpha*(u_new - thr) > 0   (alpha > 0)
    d = SA*(L1-u) + (csd + rho*t2d18) * alpha  > 0
        csd   = SA*(u - 1.8*spk - 0.01)
        t2d18 = -1.8*SA*(b - spk)
Both branches use the Sigmoid activation (no Exp) so the scalar engine
loads its activation table once.  The kernel ships the raw d scores as
bf16; the host applies (d > 0) when unpacking (sign is preserved
exactly by the bf16 downcast).

The sigmoid matmuls run in fp8e4m3 DoubleRow mode (2 contraction rows
per cycle, 2x f32r throughput): weights scaled by SW=2048, moving
operand z = SA*(L1+state) quantized on the fly by the DVE; psum is
descaled by 1/(SA*SW) = 2^-16 inside the sigmoid's activation scale.
Near the spike threshold the sigmoid-branch error is damped by
|d(u_new)/dz| <= 0.01, so fp8 there costs only ~50 extra spike flips
(budget ~1670 at rel-err 2e-2; measured v2: 368 flips total).

mm1 (L1 = W_syn @ x) precision modes (MM1_MODE env):
    f32r - TF32-like (1 cyc/row), ~1.5e-4 rel err on L1, 16MB weights
    fp16 - fp16 (1 cyc/row), ~4e-4 rel err on L1, 8MB weights

Sharding: batch 4096 -> 8 shards of 512; weights replicated; no
cross-core communication.  State tensors ship as bf16 combinations,
exact for this problem's zero-filled states.
"""

import os

import numpy as np
import ml_dtypes

import concourse.bacc as bacc
import concourse.tile as tile
import concourse.mybir as mybir
from concourse.bass_utils import run_bass_kernel_spmd

AF = mybir.ActivationFunctionType
ALU = mybir.AluOpType
PM = mybir.MatmulPerfMode

B, I, O = 4096, 2048, 2048
NCORES = 8
BC = B // NCORES          # 512 batch rows per core
P = 128                   # partitions
KT = I // P               # 16 k-tiles (mm1)
OT = O // P               # 16 output neuron tiles
KK = O // 256             # 8 double-row k-tiles (mm2/mm3)

SA = 32.0                 # global activation scale
SW = 2048.0               # fp8 weight scale
PS_INV = 1.0 / (SA * SW)  # 2^-16, exact
FP8_MAX = 240.0           # ml_dtypes.float8_e4m3 max finite

F32 = mybir.dt.float32
F32R = mybir.dt.float32r
F16 = mybir.dt.float16
BF16 = mybir.dt.bfloat16
FP8 = mybir.dt.float8e4
U8 = mybir.dt.uint8

MM1_MODE = os.environ.get("MM1_MODE", "fp16")

# phase-2 o-tile groups: pairs for wide (1024-col) DVE ops, singles at
# the end to keep the exposed pointwise tail after the last matmul short
GROUPS = [(0, 1), (2, 3), (4, 5), (6, 7), (8, 9), (10, 11), (12, 13), (14,), (15,)]


def build_nc():
    mm1_dt = {"f32r": F32R, "fp16": F16}[MM1_MODE]

    nc = bacc.Bacc("TRN2", target_bir_lowering=False, debug=False)

    x_d = nc.dram_tensor("xh", (P, KT, BC), mm1_dt, kind="ExternalInput").ap()
    u32_d = nc.dram_tensor("u32", (P, OT, BC), BF16, kind="ExternalInput").ap()
    b32_d = nc.dram_tensor("b32", (P, OT, BC), BF16, kind="ExternalInput").ap()
    t2d_d = nc.dram_tensor("t2d", (P, OT, BC), BF16, kind="ExternalInput").ap()
    csd_d = nc.dram_tensor("csd", (P, OT, BC), BF16, kind="ExternalInput").ap()
    wsyn_d = nc.dram_tensor("wsyn", (P, OT, KT, P), mm1_dt, kind="ExternalInput").ap()
    wtm_d = nc.dram_tensor("wtm", (P, OT, KK, 2, P), FP8, kind="ExternalInput").ap()
    wtadp_d = nc.dram_tensor("wtadp", (P, OT, KK, 2, P), FP8, kind="ExternalInput").ap()
    bsyn_d = nc.dram_tensor("bsyn", (P, OT), F32, kind="ExternalInput").ap()
    btm_d = nc.dram_tensor("btm", (P, OT), F32, kind="ExternalInput").ap()
    btadp_d = nc.dram_tensor("btadp", (P, OT), F32, kind="ExternalInput").ap()
    out_d = nc.dram_tensor("out", (P, OT, BC), BF16, kind="ExternalOutput").ap()

    with tile.TileContext(nc) as tc:
        with (
            tc.tile_pool(name="persist", bufs=1) as persist,
            tc.tile_pool(name="wpool", bufs=6) as wpool,
            tc.tile_pool(name="wqpool", bufs=8) as wqpool,
            tc.tile_pool(name="iopool", bufs=6) as iopool,
            tc.tile_pool(name="tmpf", bufs=4) as tmpf,
            tc.tile_pool(name="tmpb", bufs=6) as tmpb,
            tc.tile_pool(name="outp", bufs=3) as outp,
            tc.tile_pool(name="psum", bufs=4, space="PSUM") as psum,
        ):
            xsb = persist.tile([P, KT, BC], mm1_dt, tag="xsb")
            l1s = persist.tile([P, OT, BC], F32, tag="l1s")
            z1q = persist.tile([P, OT, BC], FP8, tag="z1q")
            z2q = persist.tile([P, OT, BC], FP8, tag="z2q")
            u32 = persist.tile([P, OT, BC], BF16, tag="u32")
            t1s = persist.tile([P, OT, BC], F32, tag="t1s")
            bsyn = persist.tile([P, OT], F32, tag="bsyn")
            btm = persist.tile([P, OT], F32, tag="btm")
            btadp = persist.tile([P, OT], F32, tag="btadp")

            # startup: interleave weight-tile-0 k-slices with x k-slices so
            # the first matmul chain starts after ~128KB has landed
            w0 = wpool.tile([P, KT, P], mm1_dt, tag="w")
            qh = KT // 4
            nc.sync.dma_start(w0[:, :qh, :], wsyn_d[:, 0, :qh, :])
            nc.scalar.dma_start(xsb[:, :qh, :], x_d[:, :qh, :])
            nc.sync.dma_start(w0[:, qh:, :], wsyn_d[:, 0, qh:, :])
            for c in range(1, 4):
                nc.scalar.dma_start(xsb[:, c * qh:(c + 1) * qh, :],
                                    x_d[:, c * qh:(c + 1) * qh, :])
            nc.gpsimd.dma_start(bsyn[:], bsyn_d[:])
            nc.gpsimd.dma_start(btm[:], btm_d[:])
            nc.gpsimd.dma_start(btadp[:], btadp_d[:])

            # ---- phase 1: L1 = W_syn @ x; z1 = SA*(L1+u), z2 = SA*(L1+b) fp8
            # State DMAs (u32/b32) are deferred past t=4 so the DMA-engine
            # ramp-up window feeds only weights+x (early chains were
            # DMA-starved).  z-adds for pairs 0-2 run at t=5; pair p>=3 at
            # t=2p+1.  Each pair computes z2q first so phase 2 (whose first
            # chains read z2q) is gated as little as possible by pair 7.
            h = KT // 2
            bpairs = {}

            def issue_states(s):
                nc.gpsimd.dma_start(u32[:, s, :], u32_d[:, s, :])
                p = s // 2
                if p not in bpairs:
                    bpairs[p] = iopool.tile([P, 2, BC], BF16, tag="io", name="bp")
                nc.gpsimd.dma_start(bpairs[p][:, s % 2, :], b32_d[:, s, :])

            def zadds(p):
                lo, hi = 2 * p, 2 * p + 2
                nc.vector.tensor_add(z2q[:, lo:hi, :], l1s[:, lo:hi, :],
                                     bpairs[p][:])
                nc.vector.tensor_add(z1q[:, lo:hi, :], l1s[:, lo:hi, :],
                                     u32[:, lo:hi, :])
                nc.vector.tensor_sub(t1s[:, lo:hi, :], l1s[:, lo:hi, :],
                                     u32[:, lo:hi, :])

            for t in range(OT):
                if t == 0:
                    w = w0
                else:
                    w = wpool.tile([P, KT, P], mm1_dt, tag="w")
                    nc.sync.dma_start(w[:, :h, :], wsyn_d[:, t, :h, :])
                    nc.sync.dma_start(w[:, h:, :], wsyn_d[:, t, h:, :])
                if t == 4:
                    for s in range(5):
                        issue_states(s)
                elif t > 4:
                    issue_states(t)
                if t == 10:
                    # prefetch the first two phase-2 fp8 weight groups
                    wq_pre = {}
                    for tt in (0, 1, 2, 3):
                        wr_p = wqpool.tile([P, KK, 2, P], FP8, tag="wq")
                        nc.gpsimd.dma_start(wr_p[:], wtadp_d[:, tt])
                        wa_p = wqpool.tile([P, KK, 2, P], FP8, tag="wq")
                        nc.gpsimd.dma_start(wa_p[:], wtm_d[:, tt])
                        wq_pre[tt] = (wr_p, wa_p)
                ps = psum.tile([P, 2, BC], F32, tag="ps")
                for k in range(KT):
                    nc.tensor.matmul(ps[:, 0, :], w[:, k, :], xsb[:, k, :],
                                     start=(k == 0), stop=(k == KT - 1))
                nc.scalar.activation(l1s[:, t, :], ps[:, 0, :], AF.Identity,
                                     bias=bsyn[:, t:t + 1], scale=SA)
                if t == 5:
                    zadds(0), zadds(1), zadds(2)
                elif t % 2 == 1 and t >= 7:
                    zadds((t - 1) // 2)

            # ---- phase 2: sigmoid branches (fp8 DoubleRow) + pointwise tail
            for g in GROUPS:
                gw = len(g)
                t0 = g[0]
                ws = {}
                for t in g:
                    if t in wq_pre:
                        ws[t] = wq_pre[t]
                    else:
                        wr = wqpool.tile([P, KK, 2, P], FP8, tag="wq")
                        nc.gpsimd.dma_start(wr[:], wtadp_d[:, t])
                        wa = wqpool.tile([P, KK, 2, P], FP8, tag="wq")
                        nc.gpsimd.dma_start(wa[:], wtm_d[:, t])
                        ws[t] = (wr, wa)
                t2t = iopool.tile([P, 2, BC], BF16, tag="io", name="t2t")[:, :gw, :]
                cst = iopool.tile([P, 2, BC], BF16, tag="io", name="cst")[:, :gw, :]
                nc.gpsimd.dma_start(t2t[:], t2d_d[:, t0:t0 + gw, :])
                nc.gpsimd.dma_start(cst[:], csd_d[:, t0:t0 + gw, :])

                t1 = t1s[:, t0:t0 + gw, :]

                psr = psum.tile([P, 2, BC], F32, tag="ps")
                for j, t in enumerate(g):
                    for k in range(KK):
                        nc.tensor.matmul(psr[:, j, :], ws[t][0][:, k, :, :],
                                         z2q[:, 2 * k:2 * k + 2, :],
                                         start=(k == 0), stop=(k == KK - 1),
                                         perf_mode=PM.DoubleRow)
                rho = tmpb.tile([P, 2, BC], BF16, tag="t", name="rho")[:, :gw, :]
                for j, t in enumerate(g):
                    nc.scalar.activation(rho[:, j, :], psr[:, j, :], AF.Sigmoid,
                                         bias=btadp[:, t:t + 1], scale=PS_INV)
                m2 = tmpb.tile([P, 2, BC], BF16, tag="t", name="m2")[:, :gw, :]
                nc.vector.tensor_mul(m2[:], rho[:], t2t[:])
                q = tmpb.tile([P, 2, BC], BF16, tag="t", name="q")[:, :gw, :]
                nc.vector.tensor_add(q[:], m2[:], cst[:])

                psa = psum.tile([P, 2, BC], F32, tag="ps")
                for j, t in enumerate(g):
                    for k in range(KK):
                        nc.tensor.matmul(psa[:, j, :], ws[t][1][:, k, :, :],
                                         z1q[:, 2 * k:2 * k + 2, :],
                                         start=(k == 0), stop=(k == KK - 1),
                                         perf_mode=PM.DoubleRow)
                alpha = tmpf.tile([P, 2, BC], F32, tag="t", name="alpha")[:, :gw, :]
                r = tmpf.tile([P, 2, BC], F32, tag="t", name="r")[:, :gw, :]
                dd = outp.tile([P, 2, BC], BF16, tag="o", name="dd")[:, :gw, :]
                if g is GROUPS[-1]:
                    # last tile: pipeline the exposed tail in half-columns
                    hb = BC // 2
                    for s in (slice(0, hb), slice(hb, BC)):
                        nc.scalar.activation(alpha[:, 0, s], psa[:, 0, s],
                                             AF.Sigmoid, bias=btm[:, t0:t0 + 1],
                                             scale=PS_INV)
                        nc.vector.tensor_mul(r[:, 0, s], q[:, 0, s], alpha[:, 0, s])
                        nc.vector.tensor_add(dd[:, 0, s], t1[:, 0, s], r[:, 0, s])
                        nc.sync.dma_start(out_d[:, t0, s], dd[:, 0, s])
                else:
                    for j, t in enumerate(g):
                        nc.scalar.activation(alpha[:, j, :], psa[:, j, :], AF.Sigmoid,
                                             bias=btm[:, t:t + 1], scale=PS_INV)
                    nc.vector.tensor_mul(r[:], q[:], alpha[:])
                    nc.vector.tensor_add(dd[:], t1[:], r[:])
                    nc.sync.dma_start(out_d[:, t0:t0 + gw, :], dd[:])

    nc.compile()
    return nc


def _pack_w1(w: np.ndarray) -> np.ndarray:
    # [O, I] -> [p, t, k, m] with w[t*128+m, k*128+p] at [p, t, k, m]
    return np.ascontiguousarray(w.reshape(OT, P, KT, P).transpose(3, 0, 2, 1))


def _pack_wq(w: np.ndarray) -> np.ndarray:
    # [O, O] -> [p, t, kk, i, m] fp8 with w[t*128+m, kk*256+i*128+p]*SW
    ws = np.clip(np.asarray(w, np.float64) * SW, -FP8_MAX, FP8_MAX)
    a = ws.reshape(OT, P, KK, 2, P).transpose(4, 0, 2, 3, 1)
    return np.ascontiguousarray(a.astype(ml_dtypes.float8_e4m3))


def _pack_bias(v: np.ndarray) -> np.ndarray:
    return np.ascontiguousarray(np.asarray(v, np.float32).reshape(OT, P).T)


def _pack_state(a: np.ndarray) -> np.ndarray:
    # [BC, O] -> [P, OT, BC] bf16
    return np.ascontiguousarray(
        a.reshape(BC, OT, P).transpose(2, 1, 0).astype(ml_dtypes.bfloat16))


def prepare_in_maps(x_t, u_t, b_t, spk, W_syn, b_syn, W_Tm, b_Tm, W_Tadp, b_Tadp):
    np1 = {"f32r": np.float32, "fp16": np.float16}[MM1_MODE]
    wsyn = _pack_w1(np.asarray(W_syn, np.float32)).astype(np1)
    wtm = _pack_wq(W_Tm)
    wtadp = _pack_wq(W_Tadp)
    bsyn = _pack_bias(SA * np.asarray(b_syn, np.float32))
    btm = _pack_bias(b_Tm)
    btadp = _pack_bias(b_Tadp)

    in_maps = []
    for c in range(NCORES):
        sl = slice(c * BC, (c + 1) * BC)
        xc = np.asarray(x_t[sl], np.float32)
        xp = np.ascontiguousarray(
            xc.reshape(BC, KT, P).transpose(2, 1, 0)).astype(np1)
        uc = np.asarray(u_t[sl], np.float64)
        bc = np.asarray(b_t[sl], np.float64)
        sc = np.asarray(spk[sl], np.float64)
        m = {
            "xh": xp,
            "u32": _pack_state(SA * uc),
            "b32": _pack_state(SA * bc),
            "t2d": _pack_state(-1.8 * SA * (bc - sc)),
            "csd": _pack_state(SA * (uc - 1.8 * sc - 0.01)),
            "wsyn": wsyn, "wtm": wtm, "wtadp": wtadp,
            "bsyn": bsyn, "btm": btm, "btadp": btadp,
        }
        in_maps.append(m)
    return in_maps


def unpack_output(results) -> np.ndarray:
    # per-core out: [P, OT, BC] bf16 scores -> spikes [BC, O] f32
    parts = []
    for r in results:
        d = r["out"].astype(np.float32).transpose(2, 1, 0).reshape(BC, O)
        parts.append((d > 0).astype(np.float32))
    return np.ascontiguousarray(np.concatenate(parts, axis=0))


_NC = None


def get_nc():
    global _NC
    if _NC is None:
        _NC = build_nc()
    return _NC


def run_sharded(in_maps, trace=False, **kw):
    nc = get_nc()
    return run_bass_kernel_spmd(nc, in_maps, list(range(NCORES)), trace=trace, **kw)


def kernel(**inputs) -> np.ndarray:
    in_maps = prepare_in_maps(**inputs)
    res = run_sharded(in_maps)
    return unpack_output(res.results)


# revision 16
# speedup vs baseline: 1.0805x; 1.0068x over previous
"""LSNN layer forward on 8 Trainium2 NeuronCores (data-parallel over batch).

Reference math (per batch row):
    L1    = x_t @ W_syn.T + b_syn
    alpha = sigmoid((L1 + u_t) @ W_Tm.T + b_Tm)
    rho   = sigmoid((L1 + b_t) @ W_Tadp.T + b_Tadp)
    b_new = rho * b_t + (1 - rho) * spk
    thr   = 0.01 + 1.8 * b_new
    u_new = u_t + (L1 - u_t) / alpha
    o_spk = (u_new - thr > 0) as f32

Device formulation (activations transposed [neuron, batch], all values
pre-scaled by SA=32 so the fp8 moving operand uses the e4m3 range):
    u_new - thr > 0  <=>  alpha*(u_new - thr) > 0   (alpha > 0)
    d = SA*(L1-u) + (csd + rho*t2d18) * alpha  > 0
        csd   = SA*(u - 1.8*spk - 0.01)
        t2d18 = -1.8*SA*(b - spk)
Both branches use the Sigmoid activation (no Exp) so the scalar engine
loads its activation table once.  The kernel ships the raw d scores as
bf16; the host applies (d > 0) when unpacking (sign is preserved
exactly by the bf16 downcast).

The sigmoid matmuls run in fp8e4m3 DoubleRow mode (2 contraction rows
per cycle, 2x f32r throughput): weights scaled by SW=2048, moving
operand z = SA*(L1+state) quantized on the fly by the DVE; psum is
descaled by 1/(SA*SW) = 2^-16 inside the sigmoid's activation scale.
Near the spike threshold the sigmoid-branch error is damped by
|d(u_new)/dz| <= 0.01, so fp8 there costs only ~50 extra spike flips
(budget ~1670 at rel-err 2e-2; measured v2: 368 flips total).

mm1 (L1 = W_syn @ x) precision modes (MM1_MODE env):
    f32r - TF32-like (1 cyc/row), ~1.5e-4 rel err on L1, 16MB weights
    fp16 - fp16 (1 cyc/row), ~4e-4 rel err on L1, 8MB weights (default)

Sharding: batch 4096 -> 8 shards of 512; weights replicated; no
cross-core communication.  State tensors ship as bf16 combinations,
exact for this problem's zero-filled states.

Scheduling notes (from perfetto traces of this kernel):
  - dma_start instructions cost ~0.65us EACH on the issuing engine's
    queue, so the startup uses few, large transfers and spreads issue
    across the sync (wsyn), scalar (x), and gpsimd (states, fp8
    weights) queues; with one queue the early chains are
    descriptor-starved, not bandwidth-starved.
  - u32/b32 state DMAs are deferred past t=4 so the DMA-engine ramp
    (~8us spin-up, full rate only from ~20us) feeds only weights+x.
  - z-adds run z2q first; phase 2's first chains read z2q, so the
    phase-1 -> phase-2 barrier exposes only one 1024-col DVE op.
  - GpSimd tensor ops are ~5x slower than DVE on wide f32 - keep the
    pointwise tail on vector/scalar only.
  - Per-run clock state varies (2.4 vs 2.0 GHz: matmul cadence 216ns
    vs 259ns); compare traces by cadence, not wall time.

Measured on trn2 (8 cores): 137.9-139.0us at 2.4GHz clock, 747 spike
flips (rel err 1.34e-2); baseline (f32r everywhere, Exp branch,
u8 output) was 218-234us at 322 flips.
"""

import os

import numpy as np
import ml_dtypes

import concourse.bacc as bacc
import concourse.tile as tile
import concourse.mybir as mybir
from concourse.bass_utils import run_bass_kernel_spmd

AF = mybir.ActivationFunctionType
ALU = mybir.AluOpType
PM = mybir.MatmulPerfMode

B, I, O = 4096, 2048, 2048
NCORES = 8
BC = B // NCORES          # 512 batch rows per core
P = 128                   # partitions
KT = I // P               # 16 k-tiles (mm1)
OT = O // P               # 16 output neuron tiles
KK = O // 256             # 8 double-row k-tiles (mm2/mm3)

SA = 32.0                 # global activation scale
SW = 2048.0               # fp8 weight scale
PS_INV = 1.0 / (SA * SW)  # 2^-16, exact
FP8_MAX = 240.0           # ml_dtypes.float8_e4m3 max finite

F32 = mybir.dt.float32
F32R = mybir.dt.float32r
F16 = mybir.dt.float16
BF16 = mybir.dt.bfloat16
FP8 = mybir.dt.float8e4
U8 = mybir.dt.uint8

MM1_MODE = os.environ.get("MM1_MODE", "fp16")

# phase-2 o-tile groups: pairs for wide (1024-col) DVE ops, singles at
# the end to keep the exposed pointwise tail after the last matmul short
GROUPS = [(0, 1), (2, 3), (4, 5), (6, 7), (8, 9), (10, 11), (12, 13), (14,), (15,)]


def build_nc():
    mm1_dt = {"f32r": F32R, "fp16": F16}[MM1_MODE]

    nc = bacc.Bacc("TRN2", target_bir_lowering=False, debug=False)

    x_d = nc.dram_tensor("xh", (P, KT, BC), mm1_dt, kind="ExternalInput").ap()
    u32_d = nc.dram_tensor("u32", (P, OT, BC), BF16, kind="ExternalInput").ap()
    b32_d = nc.dram_tensor("b32", (P, OT, BC), BF16, kind="ExternalInput").ap()
    t2d_d = nc.dram_tensor("t2d", (P, OT, BC), BF16, kind="ExternalInput").ap()
    csd_d = nc.dram_tensor("csd", (P, OT, BC), BF16, kind="ExternalInput").ap()
    wsyn_d = nc.dram_tensor("wsyn", (P, OT, KT, P), mm1_dt, kind="ExternalInput").ap()
    wtm_d = nc.dram_tensor("wtm", (P, OT, KK, 2, P), FP8, kind="ExternalInput").ap()
    wtadp_d = nc.dram_tensor("wtadp", (P, OT, KK, 2, P), FP8, kind="ExternalInput").ap()
    bsyn_d = nc.dram_tensor("bsyn", (P, OT), F32, kind="ExternalInput").ap()
    btm_d = nc.dram_tensor("btm", (P, OT), F32, kind="ExternalInput").ap()
    btadp_d = nc.dram_tensor("btadp", (P, OT), F32, kind="ExternalInput").ap()
    out_d = nc.dram_tensor("out", (P, OT, BC), BF16, kind="ExternalOutput").ap()

    with tile.TileContext(nc) as tc:
        with (
            tc.tile_pool(name="persist", bufs=1) as persist,
            tc.tile_pool(name="wpool", bufs=6) as wpool,
            tc.tile_pool(name="wqpool", bufs=8) as wqpool,
            tc.tile_pool(name="iopool", bufs=6) as iopool,
            tc.tile_pool(name="tmpf", bufs=4) as tmpf,
            tc.tile_pool(name="tmpb", bufs=6) as tmpb,
            tc.tile_pool(name="outp", bufs=3) as outp,
            tc.tile_pool(name="psum", bufs=4, space="PSUM") as psum,
        ):
            xsb = persist.tile([P, KT, BC], mm1_dt, tag="xsb")
            l1s = persist.tile([P, OT, BC], F32, tag="l1s")
            z1q = persist.tile([P, OT, BC], FP8, tag="z1q")
            z2q = persist.tile([P, OT, BC], FP8, tag="z2q")
            u32 = persist.tile([P, OT, BC], BF16, tag="u32")
            t1s = persist.tile([P, OT, BC], F32, tag="t1s")
            bsyn = persist.tile([P, OT], F32, tag="bsyn")
            btm = persist.tile([P, OT], F32, tag="btm")
            btadp = persist.tile([P, OT], F32, tag="btadp")

            # startup: interleave weight-tile-0 k-slices with x k-slices so
            # the first matmul chain starts after ~128KB has landed
            w0 = wpool.tile([P, KT, P], mm1_dt, tag="w")
            qh = KT // 4
            nc.sync.dma_start(w0[:, :qh, :], wsyn_d[:, 0, :qh, :])
            nc.scalar.dma_start(xsb[:, :qh, :], x_d[:, :qh, :])
            nc.sync.dma_start(w0[:, qh:, :], wsyn_d[:, 0, qh:, :])
            for c in range(1, 4):
                nc.scalar.dma_start(xsb[:, c * qh:(c + 1) * qh, :],
                                    x_d[:, c * qh:(c + 1) * qh, :])
            nc.gpsimd.dma_start(bsyn[:], bsyn_d[:])
            nc.gpsimd.dma_start(btm[:], btm_d[:])
            nc.gpsimd.dma_start(btadp[:], btadp_d[:])

            # ---- phase 1: L1 = W_syn @ x; z1 = SA*(L1+u), z2 = SA*(L1+b) fp8
            # State DMAs (u32/b32) are deferred past t=4 so the DMA-engine
            # ramp-up window feeds only weights+x (early chains were
            # DMA-starved).  z-adds for pairs 0-2 run at t=5; pair p>=3 at
            # t=2p+1.  Each pair computes z2q first so phase 2 (whose first
            # chains read z2q) is gated as little as possible by pair 7.
            h = KT // 2
            bpairs = {}

            def issue_states(s):
                nc.gpsimd.dma_start(u32[:, s, :], u32_d[:, s, :])
                p = s // 2
                if p not in bpairs:
                    bpairs[p] = iopool.tile([P, 2, BC], BF16, tag="io", name="bp")
                nc.gpsimd.dma_start(bpairs[p][:, s % 2, :], b32_d[:, s, :])

            def zadds(p):
                lo, hi = 2 * p, 2 * p + 2
                nc.vector.tensor_add(z2q[:, lo:hi, :], l1s[:, lo:hi, :],
                                     bpairs[p][:])
                nc.vector.tensor_add(z1q[:, lo:hi, :], l1s[:, lo:hi, :],
                                     u32[:, lo:hi, :])
                nc.vector.tensor_sub(t1s[:, lo:hi, :], l1s[:, lo:hi, :],
                                     u32[:, lo:hi, :])

            for t in range(OT):
                if t == 0:
                    w = w0
                else:
                    w = wpool.tile([P, KT, P], mm1_dt, tag="w")
                    nc.sync.dma_start(w[:, :h, :], wsyn_d[:, t, :h, :])
                    nc.sync.dma_start(w[:, h:, :], wsyn_d[:, t, h:, :])
                if t == 4:
                    for s in range(5):
                        issue_states(s)
                elif t > 4:
                    issue_states(t)
                if t == 10:
                    # prefetch the first two phase-2 fp8 weight groups
                    wq_pre = {}
                    for tt in (0, 1, 2, 3):
                        wr_p = wqpool.tile([P, KK, 2, P], FP8, tag="wq")
                        nc.gpsimd.dma_start(wr_p[:], wtadp_d[:, tt])
                        wa_p = wqpool.tile([P, KK, 2, P], FP8, tag="wq")
                        nc.gpsimd.dma_start(wa_p[:], wtm_d[:, tt])
                        wq_pre[tt] = (wr_p, wa_p)
                ps = psum.tile([P, 2, BC], F32, tag="ps")
                for k in range(KT):
                    nc.tensor.matmul(ps[:, 0, :], w[:, k, :], xsb[:, k, :],
                                     start=(k == 0), stop=(k == KT - 1))
                nc.scalar.activation(l1s[:, t, :], ps[:, 0, :], AF.Identity,
                                     bias=bsyn[:, t:t + 1], scale=SA)
                if t == 5:
                    zadds(0), zadds(1), zadds(2)
                elif t % 2 == 1 and t >= 7:
                    zadds((t - 1) // 2)

            # ---- phase 2: sigmoid branches (fp8 DoubleRow) + pointwise tail
            for g in GROUPS:
                gw = len(g)
                t0 = g[0]
                ws = {}
                for t in g:
                    if t in wq_pre:
                        ws[t] = wq_pre[t]
                    else:
                        wr = wqpool.tile([P, KK, 2, P], FP8, tag="wq")
                        nc.gpsimd.dma_start(wr[:], wtadp_d[:, t])
                        wa = wqpool.tile([P, KK, 2, P], FP8, tag="wq")
                        nc.gpsimd.dma_start(wa[:], wtm_d[:, t])
                        ws[t] = (wr, wa)
                t2t = iopool.tile([P, 2, BC], BF16, tag="io", name="t2t")[:, :gw, :]
                cst = iopool.tile([P, 2, BC], BF16, tag="io", name="cst")[:, :gw, :]
                nc.gpsimd.dma_start(t2t[:], t2d_d[:, t0:t0 + gw, :])
                nc.gpsimd.dma_start(cst[:], csd_d[:, t0:t0 + gw, :])

                t1 = t1s[:, t0:t0 + gw, :]

                psr = psum.tile([P, 2, BC], F32, tag="ps")
                for j, t in enumerate(g):
                    for k in range(KK):
                        nc.tensor.matmul(psr[:, j, :], ws[t][0][:, k, :, :],
                                         z2q[:, 2 * k:2 * k + 2, :],
                                         start=(k == 0), stop=(k == KK - 1),
                                         perf_mode=PM.DoubleRow)
                rho = tmpb.tile([P, 2, BC], BF16, tag="t", name="rho")[:, :gw, :]
                for j, t in enumerate(g):
                    nc.scalar.activation(rho[:, j, :], psr[:, j, :], AF.Sigmoid,
                                         bias=btadp[:, t:t + 1], scale=PS_INV)
                m2 = tmpb.tile([P, 2, BC], BF16, tag="t", name="m2")[:, :gw, :]
                nc.vector.tensor_mul(m2[:], rho[:], t2t[:])
                q = tmpb.tile([P, 2, BC], BF16, tag="t", name="q")[:, :gw, :]
                nc.vector.tensor_add(q[:], m2[:], cst[:])

                psa = psum.tile([P, 2, BC], F32, tag="ps")
                for j, t in enumerate(g):
                    for k in range(KK):
                        nc.tensor.matmul(psa[:, j, :], ws[t][1][:, k, :, :],
                                         z1q[:, 2 * k:2 * k + 2, :],
                                         start=(k == 0), stop=(k == KK - 1),
                                         perf_mode=PM.DoubleRow)
                alpha = tmpf.tile([P, 2, BC], F32, tag="t", name="alpha")[:, :gw, :]
                r = tmpf.tile([P, 2, BC], F32, tag="t", name="r")[:, :gw, :]
                dd = outp.tile([P, 2, BC], BF16, tag="o", name="dd")[:, :gw, :]
                if g is GROUPS[-1]:
                    # last tile: pipeline the exposed tail in half-columns
                    hb = BC // 2
                    for s in (slice(0, hb), slice(hb, BC)):
                        nc.scalar.activation(alpha[:, 0, s], psa[:, 0, s],
                                             AF.Sigmoid, bias=btm[:, t0:t0 + 1],
                                             scale=PS_INV)
                        nc.vector.tensor_mul(r[:, 0, s], q[:, 0, s], alpha[:, 0, s])
                        nc.vector.tensor_add(dd[:, 0, s], t1[:, 0, s], r[:, 0, s])
                        nc.sync.dma_start(out_d[:, t0, s], dd[:, 0, s])
                else:
                    for j, t in enumerate(g):
                        nc.scalar.activation(alpha[:, j, :], psa[:, j, :], AF.Sigmoid,
                                             bias=btm[:, t:t + 1], scale=PS_INV)
                    nc.vector.tensor_mul(r[:], q[:], alpha[:])
                    nc.vector.tensor_add(dd[:], t1[:], r[:])
                    nc.sync.dma_start(out_d[:, t0:t0 + gw, :], dd[:])

    nc.compile()
    return nc


def _pack_w1(w: np.ndarray) -> np.ndarray:
    # [O, I] -> [p, t, k, m] with w[t*128+m, k*128+p] at [p, t, k, m]
    return np.ascontiguousarray(w.reshape(OT, P, KT, P).transpose(3, 0, 2, 1))


def _pack_wq(w: np.ndarray) -> np.ndarray:
    # [O, O] -> [p, t, kk, i, m] fp8 with w[t*128+m, kk*256+i*128+p]*SW
    ws = np.clip(np.asarray(w, np.float64) * SW, -FP8_MAX, FP8_MAX)
    a = ws.reshape(OT, P, KK, 2, P).transpose(4, 0, 2, 3, 1)
    return np.ascontiguousarray(a.astype(ml_dtypes.float8_e4m3))


def _pack_bias(v: np.ndarray) -> np.ndarray:
    return np.ascontiguousarray(np.asarray(v, np.float32).reshape(OT, P).T)


def _pack_state(a: np.ndarray) -> np.ndarray:
    # [BC, O] -> [P, OT, BC] bf16
    return np.ascontiguousarray(
        a.reshape(BC, OT, P).transpose(2, 1, 0).astype(ml_dtypes.bfloat16))


def prepare_in_maps(x_t, u_t, b_t, spk, W_syn, b_syn, W_Tm, b_Tm, W_Tadp, b_Tadp):
    np1 = {"f32r": np.float32, "fp16": np.float16}[MM1_MODE]
    wsyn = _pack_w1(np.asarray(W_syn, np.float32)).astype(np1)
    wtm = _pack_wq(W_Tm)
    wtadp = _pack_wq(W_Tadp)
    bsyn = _pack_bias(SA * np.asarray(b_syn, np.float32))
    btm = _pack_bias(b_Tm)
    btadp = _pack_bias(b_Tadp)

    in_maps = []
    for c in range(NCORES):
        sl = slice(c * BC, (c + 1) * BC)
        xc = np.asarray(x_t[sl], np.float32)
        xp = np.ascontiguousarray(
            xc.reshape(BC, KT, P).transpose(2, 1, 0)).astype(np1)
        uc = np.asarray(u_t[sl], np.float64)
        bc = np.asarray(b_t[sl], np.float64)
        sc = np.asarray(spk[sl], np.float64)
        m = {
            "xh": xp,
            "u32": _pack_state(SA * uc),
            "b32": _pack_state(SA * bc),
            "t2d": _pack_state(-1.8 * SA * (bc - sc)),
            "csd": _pack_state(SA * (uc - 1.8 * sc - 0.01)),
            "wsyn": wsyn, "wtm": wtm, "wtadp": wtadp,
            "bsyn": bsyn, "btm": btm, "btadp": btadp,
        }
        in_maps.append(m)
    return in_maps


def unpack_output(results) -> np.ndarray:
    # per-core out: [P, OT, BC] bf16 scores -> spikes [BC, O] f32
    parts = []
    for r in results:
        d = r["out"].astype(np.float32).transpose(2, 1, 0).reshape(BC, O)
        parts.append((d > 0).astype(np.float32))
    return np.ascontiguousarray(np.concatenate(parts, axis=0))


_NC = None


def get_nc():
    global _NC
    if _NC is None:
        _NC = build_nc()
    return _NC


def run_sharded(in_maps, trace=False, **kw):
    nc = get_nc()
    return run_bass_kernel_spmd(nc, in_maps, list(range(NCORES)), trace=trace, **kw)


def kernel(**inputs) -> np.ndarray:
    in_maps = prepare_in_maps(**inputs)
    res = run_sharded(in_maps)
    return unpack_output(res.results)
